# revision 23
# baseline (speedup 1.0000x reference)
"""Trainium2 Bass kernel for nn_GATrAutoRegressorLoss (v3).

Strategy (data-parallel over the hit axis N, 8 cores):
  - The dominant cost is the assignment BCE over (T=32, N=500000) logits.
    Only ~half the (t, hit) pairs are valid (t < cmin[hit]); the invalid
    ones contribute exactly 0.  The host compacts the valid logits into a
    dense fp8 stream (pad = -96, softplus underflows to exactly 0), sharded
    evenly across 8 cores as (128, CAP) tiles.
  - Per core the stream is chunked.  ACT computes u = exp(x) (fp8 in, bf16
    out; fp8 input runs at the same 0.87 ns/col rate).  For the leading
    chunks the idle DVE then computes w = 1+u (tensor_scalar, 4x mode) and
    three levels of pairwise products of contiguous halves (tensor_tensor,
    2x mode), shrinking the ACT ln pass 8x: sum ln(1+u_i) = ln(prod w_i).
    The trailing chunks take the plain ln(1+u) path on ACT so the final ln
    never waits on the DVE pipeline tail.  Products of 8 w's stay < 7e19.
  - Exp/Ln are pinned to the one ACT table containing both, loaded once.
  - The "- x*z" BCE term touches only N scattered elements; it is exact
    index bookkeeping, done on host in float64 (like the gt scatter planes).
  - The small (T,B) losses (dir/mag/pid/charge/stop) are computed on-device
    from host-scattered dense planes, column-sharded 8 ways; the 736 plane
    bytes per partition ride in front of chunk0's fp8 DMA and are bitcast
    back to f32 on SBUF, so no separate small-packet DMA exists.  Their
    DVE work is split into an early block (runs while ACT does exp0) and a
    late block (runs in the final-ln window).
  - All per-core partial sums live in one (128,16) f32 tile, reduced
    across partitions by the idle PE (ones-vector matmul) so the result
    DMA back to HBM is a single 64-byte packet.
"""

import numpy as np

import concourse.bacc as bacc
import concourse.mybir as mybir
from concourse.tile import TileContext
from concourse.bass_utils import run_bass_kernel_spmd

F32 = mybir.dt.float32
BF16 = mybir.dt.bfloat16
F8 = mybir.dt.float8e4
NP_F8 = mybir.dt.np(F8)

T, B, N, NPFO = 32, 256, 500000, 4096
L_DIR, L_MAG, L_PID, L_CHG, L_ASN, L_STP = 1.0, 1.0, 1.0, 0.5, 1.0, 0.5

N_CORES = 8
P = 128
PEN = -96.0   # pad/mask value; exp(-96) underflows to exactly 0
CHW = 2048    # chunk width (all chunks paired)

_PLANES = [
    "pm0", "pm1", "pm2", "gm0", "gm1", "gm2",
    "pp", "gp", "pch", "gch",
    "valid", "stopz",
    "poh0", "poh1", "poh2", "poh3", "poh4",
    "pid0", "pid1", "pid2", "pid3", "pid4", "stopx",
]
NPL = len(_PLANES)
SW = 8  # per-core plane width (64 total / 8 cores)

_nc_cache = {}
last_result = None


def _install_axon_hooks_shim():
    """Some images lack antenv.axon_hooks; register an equivalent backed by
    trn_agent_boot's ctypes NTFF profiler so BASS_TRACE keeps working."""
    try:
        import antenv.axon_hooks  # noqa: F401
        return
    except ImportError:
        pass
    try:
        import sys
        import types

        import antenv

        mod = types.ModuleType("antenv.axon_hooks")
        _hook = [None]

        def set_axon_ntff_profile_hook(h):
            _hook[0] = h

        def get_axon_ntff_profile_hook():
            if _hook[0] is None:
                try:
                    from trn_agent_boot.trn_boot import _ntff_profile_via_ctypes

                    _hook[0] = _ntff_profile_via_ctypes(
                        "/opt/axon/libaxon_pjrt.so"
                    )
                except Exception:
                    return None
            return _hook[0]

        mod.set_axon_ntff_profile_hook = set_axon_ntff_profile_hook
        mod.get_axon_ntff_profile_hook = get_axon_ntff_profile_hook
        sys.modules["antenv.axon_hooks"] = mod
        antenv.axon_hooks = mod
    except Exception:
        pass


_install_axon_hooks_shim()


class _Bacc(bacc.Bacc):
    """Bacc whose ACT-table chooser binds Exp/Ln to the one json table that
    contains both (natural_log_exp_and_others), so the Scalar engine loads
    its function table exactly once."""

    def insert_act_table_loads(self):
        # a manual InstLoadActFuncSet for natural_log_exp_and_others is
        # placed at program start (before the TileContext entry sync), so
        # the automatic insertion pass is skipped entirely
        return


def _chunks(cap):
    """First chunk small (512) so exp0 starts as early as possible, middle
    chunks CHW, a 1024 next-to-last, last chunk 512 taking the plain
    exp+ln path (no DVE pairing tail behind the final ln)."""
    widths = [512, 1536]
    middle = cap - 512 - 1536 - 1024 - 512
    widths += [CHW] * (middle // CHW)
    if middle % CHW:
        widths.append(middle % CHW)
    widths += [1024, 512]
    assert sum(widths) == cap and all(w % 8 == 0 for w in widths)
    ch = []
    c0 = 0
    for i, w in enumerate(widths):
        ch.append((c0, w, i < len(widths) - 1))
        c0 += w
    return ch


SMB = NPL * SW * 4  # small-loss plane bytes per partition row


def _gen(cap):
    import concourse.bass as _bass_mod

    ch = _chunks(cap)
    p3w = 0
    for i, (c0, w, p) in enumerate(ch):
        if p:
            p3w += w // 4 if i == len(ch) - 2 else w // 8
    _orig_barrier = _bass_mod.Bass.all_engine_barrier
    _bass_mod.Bass.all_engine_barrier = (
        lambda self, *, sem_only=False: None
    )
    try:
        nc = _Bacc(None, target_bir_lowering=False, debug=True)
    finally:
        _bass_mod.Bass.all_engine_barrier = _orig_barrier
    xq = nc.dram_tensor("xq", [P, SMB + cap], F8, kind="ExternalInput")
    partials = nc.dram_tensor("partials", [1, 16], F32, kind="ExternalOutput")

    AF = mybir.ActivationFunctionType
    OP = mybir.AluOpType

    # ---- pre-TileContext prologue work, all on the ACT engine: the
    # function-table load, then chunk0's DMA on ACT's own hardware DGE
    # queue (separate from sync's queue, so c1.. start sooner there too),
    # then explicit waits so every in-context read of xt0 is ordered on
    # the consuming engines.  This pulls exp0 well before the point the
    # sync-queue chain could deliver chunk0.
    w0 = ch[0][1]
    xt0h = nc.alloc_sbuf_tensor("xt0pre", [P, SMB + w0], F8)
    xt0 = xt0h.ap()
    psem = nc.alloc_semaphore("pre_dma")
    from concourse.hw_specs import get_activation_tables

    _tid = list(get_activation_tables(nc.m.arch)).index(
        "natural_log_exp_and_others"
    )
    nc.scalar.add_instruction(
        mybir.InstLoadActFuncSet(
            name=nc.get_next_instruction_name(), ins=[], outs=[],
            act_func_set_id=_tid,
        )
    )
    nc.scalar.dma_start(out=xt0[:, : SMB + w0],
                        in_=xq[:, 0 : SMB + w0]).then_inc(psem, 16)
    nc.scalar.wait_ge(psem, 16)
    nc.vector.wait_ge(psem, 16)

    with TileContext(nc) as tc:
        with (
            tc.tile_pool(name="cst", bufs=1) as cst,
            tc.tile_pool(name="io", bufs=6) as io,
            tc.tile_pool(name="wk", bufs=4) as wk,
            tc.tile_pool(name="sml", bufs=1) as sml,
            tc.tile_pool(name="ps", bufs=1, space="PSUM") as ps,
        ):
            accT = cst.tile([P, 16], F32)
            accA = accT[:, 0:8]
            accS = accT[:, 8:16]
            epsb = cst.tile([P, 1], F32)
            nc.vector.memset(epsb[:], 1e-30)
            ones = cst.tile([P, 1], F32)
            nc.vector.memset(ones[:], 1.0)
            p3b = cst.tile([P, p3w], BF16)
            lnout = cst.tile([P, p3w], BF16)

            # ---- main stream DMAs first; chunk0 carries the small-loss
            # planes (SMB raw bytes per row) in front of its stream data.
            # Chunk0's trigger rides the ACT engine (also a HWDGE engine,
            # ready at the same time) so it precedes the other triggers.
            xts = [xt0]
            for ci, (c0, w, paired) in enumerate(ch):
                if ci == 0:
                    continue
                xt = io.tile([P, CHW], F8, tag="xt")
                nc.sync.dma_start(out=xt[:, :w],
                                  in_=xq[:, SMB + c0 : SMB + c0 + w])
                xts.append(xt)
            smt = xt0[:, 0:SMB].bitcast(F32)

            # ---- small-loss plumbing
            PLI = {n: i for i, n in enumerate(_PLANES)}

            def reg(name, k=1):
                i = PLI[name]
                return smt[:, i * SW : (i + k) * SW]

            _tn = [0]

            def tmp(w_=SW):
                _tn[0] += 1
                nm = f"tmp{_tn[0]}"
                return sml.tile([P, w_], F32, name=nm, tag=nm)

            def red(out_ap, in_ap, k):
                nc.vector.tensor_reduce(
                    out=out_ap,
                    in_=in_ap.rearrange("p (k j) -> p j k", k=k),
                    axis=mybir.AxisListType.X,
                    op=OP.add,
                )

            # ---- early small-loss block: DVE-only, needs just smt.
            # Runs while ACT does the table load + exp0.
            valid = reg("valid")
            sq = tmp(6 * SW)
            nc.vector.tensor_mul(sq[:], reg("pm0", 6), reg("pm0", 6))
            ssb = tmp(2 * SW)
            red(ssb[:, 0:SW], sq[:, 0 : 3 * SW], 3)
            red(ssb[:, SW : 2 * SW], sq[:, 3 * SW : 6 * SW], 3)
            ulb = tmp(3 * SW)
            nc.vector.tensor_mul(ulb[:, 0:SW], ssb[:, 0:SW],
                                 ssb[:, SW : 2 * SW])
            dmul = tmp(3 * SW)
            nc.vector.tensor_mul(dmul[:], reg("pm0", 3), reg("gm0", 3))
            dot = tmp()
            red(dot[:], dmul[:], 3)
            dif = tmp(2 * SW)
            nc.vector.tensor_sub(dif[:, 0:SW], reg("pp"), reg("gp"))
            nc.vector.tensor_sub(dif[:, SW : 2 * SW], reg("pch"), reg("gch"))
            dsq = tmp(2 * SW)
            nc.vector.tensor_mul(dsq[:], dif[:], dif[:])
            xm = tmp(5 * SW)
            nc.vector.tensor_mul(xm[:], reg("pid0", 5), reg("poh0", 5))
            xcls = tmp()
            red(xcls[:], xm[:], 5)
            xz = tmp()
            nc.vector.tensor_mul(xz[:], reg("stopx"), reg("stopz"))

            # ---- main loop
            p3o = 0
            for ci, (c0, w, paired) in enumerate(ch):
                xt = xts[ci]
                xs = xt[:, SMB : SMB + w] if ci == 0 else xt[:, :w]
                if ci == 0:
                    # pid+stop exp first on ACT: same chunk0 gate as exp0,
                    # so it fills the pre-exp0 DMA wait instead of sitting
                    # between exp0 and exp1; its 5-group reduce runs on DVE
                    # before the pairing queue builds up.
                    pexp = tmp(6 * SW)
                    nc.scalar.activation(out=pexp[:], in_=reg("pid0", 6),
                                         func=AF.Exp)
                    red(ulb[:, SW : 2 * SW], pexp[:, 0 : 5 * SW], 5)
                if paired:
                    # the chunk right before the plain one pairs one level
                    # shallower so the final ln is gated less by the DVE
                    depth = 2 if ci == len(ch) - 2 else 3
                    ut = wk.tile([P, CHW], BF16, tag="ut")
                    nc.scalar.activation(out=ut[:, :w], in_=xs, func=AF.Exp)
                    wt = wk.tile([P, CHW], BF16, tag="wt")
                    nc.vector.tensor_scalar_add(wt[:, :w], ut[:, :w], 1.0)
                    h = w // 2
                    q1 = wk.tile([P, CHW // 2], BF16, tag="q1")
                    nc.vector.tensor_mul(q1[:, :h], wt[:, :h],
                                         wt[:, h : 2 * h])
                    h2 = h // 2
                    if depth == 2:
                        nc.vector.tensor_mul(
                            p3b[:, p3o : p3o + h2], q1[:, :h2],
                            q1[:, h2 : 2 * h2],
                        )
                        p3o += h2
                        continue
                    q2 = wk.tile([P, CHW // 4], BF16, tag="q2")
                    nc.vector.tensor_mul(q2[:, :h2], q1[:, :h2],
                                         q1[:, h2 : 2 * h2])
                    if False:
                        pass
                    else:
                        h3 = h2 // 2
                        nc.vector.tensor_mul(
                            p3b[:, p3o : p3o + h3], q2[:, :h3],
                            q2[:, h3 : 2 * h3],
                        )
                        p3o += h3
                else:
                    ut = wk.tile([P, 1024], BF16, tag="utp")
                    nc.scalar.activation(out=ut[:, :w], in_=xs, func=AF.Exp)
                    st = wk.tile([P, 1024], BF16, tag="stp")
                    nc.scalar.activation(
                        out=st[:, :w], in_=ut[:, :w], func=AF.Ln, bias=1.0,
                        accum_out=accA[:, 1:2],
                    )
                    nc.vector.tensor_scalar_add(
                        ulb[:, 2 * SW : 3 * SW], pexp[:, 5 * SW : 6 * SW],
                        1.0,
                    )

            # ---- late small-loss block: the 4 transcendental ACT ops plus
            # the DVE tail that consumes them; overlaps the final-ln window.
            lnv = tmp(3 * SW)
            nc.scalar.activation(out=lnv[:], in_=ulb[:], func=AF.Ln,
                                 bias=epsb[:])
            rsq = tmp()
            nc.scalar.activation(out=rsq[:], in_=lnv[:, 0:SW], func=AF.Exp,
                                 scale=-0.5)
            spv = lnv[:, 2 * SW : 3 * SW]

            # final ln over all chunks' grouped products
            nc.scalar.activation(
                out=lnout[:], in_=p3b[:], func=AF.Ln,
                accum_out=accA[:, 0:1],
            )

            nc.vector.tensor_mul(dot[:], dot[:], rsq[:])
            cv = tmp()
            nc.vector.tensor_mul(cv[:], dot[:], valid)
            o1 = tmp()
            nc.vector.scalar_tensor_tensor(
                out=o1[:], in0=cv[:], scalar=-1.0, in1=valid,
                op0=OP.mult, op1=OP.add, accum_out=accS[:, 0:1],
            )
            for col, sl in ((1, slice(0, SW)), (2, slice(SW, 2 * SW))):
                o = tmp()
                nc.vector.scalar_tensor_tensor(
                    out=o[:], in0=dsq[:, sl], scalar=1.0, in1=valid,
                    op0=OP.mult, op1=OP.mult,
                    accum_out=accS[:, col : col + 1],
                )
            u1 = tmp()
            nc.vector.scalar_tensor_tensor(
                out=u1[:], in0=xcls[:], scalar=-1.0,
                in1=lnv[:, SW : 2 * SW], op0=OP.mult, op1=OP.add,
            )
            o2 = tmp()
            nc.vector.scalar_tensor_tensor(
                out=o2[:], in0=u1[:], scalar=1.0, in1=valid,
                op0=OP.mult, op1=OP.mult, accum_out=accS[:, 3:4],
            )
            o3 = tmp()
            nc.vector.scalar_tensor_tensor(
                out=o3[:], in0=xz[:], scalar=-1.0, in1=spv,
                op0=OP.mult, op1=OP.add, accum_out=accS[:, 4:5],
            )
            # cross-partition reduction on the idle PE: ones.T @ accT
            pt = ps.tile([1, 16], F32)
            nc.tensor.matmul(pt[:], ones[:], accT[:], start=True, stop=True)
            outs = sml.tile([1, 16], F32)
            nc.scalar.copy(out=outs[:], in_=pt[:])
            nc.sync.dma_start(out=partials[:], in_=outs[:])
    nc.finalize()
    return nc


def _get_nc(cap):
    if cap not in _nc_cache:
        _nc_cache[cap] = _gen(cap)
    return _nc_cache[cap]


def _cumcount(gb):
    n = gb.shape[0]
    order = np.argsort(gb, kind="stable")
    sb = gb[order]
    first = np.searchsorted(sb, sb, side="left")
    cum = np.arange(n) - first
    out = np.zeros(n, dtype=np.int64)
    out[order] = cum
    return out


def kernel(**inputs):
    pfo_momentum = np.asarray(inputs["pfo_momentum"], np.float32)
    pfo_p_mod = np.asarray(inputs["pfo_p_mod"], np.float32)
    pfo_pid = np.asarray(inputs["pfo_pid"], np.float32)
    pfo_charge = np.asarray(inputs["pfo_charge"], np.float32)
    al = np.asarray(inputs["assignments_logits"], np.float32).reshape(T, N)
    stop_logits = np.asarray(inputs["stop_logits"], np.float32)
    gt_momentum = np.asarray(inputs["gt_momentum"], np.float32)
    gt_p_mod = np.asarray(inputs["gt_p_mod"], np.float32)
    gt_pid = np.asarray(inputs["gt_pid"], np.float32)
    gt_charge = np.asarray(inputs["gt_charge"], np.float32)
    gt_batch = np.asarray(inputs["gt_batch"]).astype(np.int64)
    hit_to_pfo = np.asarray(inputs["hit_to_pfo"]).astype(np.int64)
    hit_batch = np.asarray(inputs["hit_batch"]).astype(np.int64)

    # ---- host index bookkeeping ----
    ppe = np.bincount(gt_batch, minlength=B)[:B]                  # (B,)
    cmin = np.minimum(ppe[hit_batch], T).astype(np.int64)         # (N,)
    w = hit_to_pfo < cmin                                         # (N,) bool
    assign_den = max(float(cmin.sum()), 1.0)

    # exact "- x*z" term: x at (pfo(hit), hit) for valid selected hits
    b_sum = float(
        al[hit_to_pfo[w], np.flatnonzero(w)].astype(np.float64).sum()
    )

    # compact the valid logits (t < cmin[hit]) into a dense fp8 stream
    vmask = np.arange(T, dtype=np.int64)[:, None] < cmin[None, :]  # (T,N)
    vals = al[vmask]                                               # (V,) f32
    V = vals.shape[0]
    cols = -(-V // (N_CORES * P))
    cap = max(-(-cols // 1024) * 1024, 4096)
    buf = np.full(N_CORES * P * cap, PEN, np.float32)
    buf[:V] = vals
    xq_all = buf.astype(NP_F8).reshape(N_CORES, P, cap)

    step_idx = _cumcount(gt_batch)
    keep = step_idx < T
    si, gb = step_idx[keep], gt_batch[keep]

    def scat(v):
        out = np.zeros((T, B) + v.shape[1:], np.float32)
        out[si, gb] = v[keep]
        return out

    gt_mom_tb = scat(gt_momentum)
    gt_pmod_tb = scat(gt_p_mod)
    gt_pid_tb = scat(gt_pid)
    gt_chg_tb = scat(gt_charge)

    steps = np.arange(T)[:, None]
    valid = (steps < ppe[None, :]).astype(np.float32)             # (T,B)
    vcnt = max(float(valid.sum()), 1.0)
    gt_stop = (steps >= ppe[None, :]).astype(np.float32)
    gt_cls = np.argmax(gt_pid_tb, axis=-1)                        # (T,B)
    poh = np.zeros((T, B, 5), np.float32)
    np.put_along_axis(poh, gt_cls[..., None], 1.0, axis=-1)

    planes = {
        "pm0": pfo_momentum[..., 0], "pm1": pfo_momentum[..., 1],
        "pm2": pfo_momentum[..., 2],
        "gm0": gt_mom_tb[..., 0], "gm1": gt_mom_tb[..., 1],
        "gm2": gt_mom_tb[..., 2],
        "pp": pfo_p_mod[..., 0], "gp": gt_pmod_tb[..., 0],
        "pch": pfo_charge[..., 0], "gch": gt_chg_tb[..., 0],
        "stopx": stop_logits[..., 0], "stopz": gt_stop,
        "valid": valid,
        **{f"pid{k}": pfo_pid[..., k] for k in range(5)},
        **{f"poh{k}": poh[..., k] for k in range(5)},
    }
    pl64 = np.stack(
        [np.ascontiguousarray(planes[n].reshape(P, 64)) for n in _PLANES]
    )  # (NPL, P, 64)

    in_maps = []
    for c in range(N_CORES):
        smc = np.ascontiguousarray(
            pl64[:, :, c * SW : (c + 1) * SW].transpose(1, 0, 2).reshape(
                P, NPL * SW
            )
        )
        smb = smc.view(np.uint8).reshape(P, SMB).view(NP_F8)
        xq_c = np.concatenate([smb, xq_all[c]], axis=1)
        in_maps.append({"xq": np.ascontiguousarray(xq_c)})

    nc = _get_nc(cap)
    res = run_bass_kernel_spmd(nc, in_maps, core_ids=list(range(N_CORES)))
    global last_result
    last_result = res

    # ---- host combine (float64) ----
    A_sum = 0.0
    accs = np.zeros(8, np.float64)
    for c in range(N_CORES):
        pr = res.results[c]["partials"].astype(np.float64)
        A_sum += pr[0, 0] + pr[0, 1]
        accs += pr[0, 8:16]
    loss_assign = (A_sum - b_sum) / assign_den

    loss_dir = accs[0] / vcnt
    loss_mag = accs[1] / vcnt
    loss_chg = accs[2] / vcnt
    loss_pid = accs[3] / vcnt
    loss_stop = accs[4] / (T * B)

    total = (L_DIR * loss_dir + L_MAG * loss_mag + L_PID * loss_pid
             + L_CHG * loss_chg + L_ASN * loss_assign + L_STP * loss_stop)
    f = np.float32
    return (f(total), f(loss_dir), f(loss_mag), f(loss_pid), f(loss_chg),
            f(loss_assign), f(loss_stop))
